# revision 5
# baseline (speedup 1.0000x reference)
"""Causal multi-head attention block (qkv proj + causal softmax attention + out proj)
for Trainium2, sharded over 8 NeuronCores: data-parallel over batch (2) x
tensor-parallel over heads (4 heads per core of 16).

Each core computes, for its batch b and its 4 heads:
  qT,kT [hd, S] and v [S, hd]  (qkv projection, weights pre-transposed on host)
  ST    [k, 2, q] = scores of a head pair, causal-blocked (one 2-bank PSUM tile)
  P     = exp(ST + mask)   (one ACT instruction per k-tile covers both heads)
  attnT [hd+1, q] = [v | 1].T @ P   (row hd = softmax denominator)
  an    = attnT / denom
  out_partial [S, D] = an.T @ owT  (row-parallel out proj)
Host sums the 4 per-core partials of each batch.

Emission is interleaved so input DMA, qkv projection, attention and
out-projection overlap: x is DMA'd in 512-column chunks and each qkv chunk
is computed as its chunk lands; attention chunk qc is emitted as soon as
its kT/qT/v tiles exist; out-proj of chunk qc is deferred past the next
attention chunk so its matmuls never wait on the normalize chain; the
chunk order ends on the shortest chunk (qc=0) to minimize the tail.
"""

import contextlib
import sys

import numpy as np

sys.path.insert(0, "/opt/trn_rl_repo")

import concourse.bass as bass
import concourse.tile as tile
from concourse import bacc, mybir
from concourse.bass import MemorySpace
from concourse.bass_utils import run_bass_kernel_spmd

F32 = mybir.dt.float32
BF16 = mybir.dt.bfloat16
EXP = mybir.ActivationFunctionType.Exp

B, S, D = 2, 2048, 1024
H, HD = 16, 64
NCORES = 8
NH = 4          # heads per core
NP = 2          # head pairs per core
SCALE = HD ** -0.5

N_DT = D // 128          # 8 d-tiles of 128
N_ST = S // 128          # 16 seq tiles of 128
N_CH = S // 512          # 4 seq chunks of 512
FQK = 2 * NH * HD // 128  # 4 f-tiles covering q|k (pair-major)
VW = NH * HD             # 256 v columns
NEG = -1.0e9
N_WARM = 16


def _emit(tc, nc, xT_d, wT_d, owT_d, mask_d, out_d):
    ctx = contextlib.ExitStack()
    with ctx:
        # ------------- pools -------------
        sb = ctx.enter_context(tc.tile_pool(name="sb", bufs=1))
        p_pool = ctx.enter_context(tc.tile_pool(name="psb", bufs=3))
        an_pool = ctx.enter_context(tc.tile_pool(name="attn_n", bufs=2))
        sm_pool = ctx.enter_context(tc.tile_pool(name="smalls", bufs=4))
        bc_pool = ctx.enter_context(tc.tile_pool(name="bcast", bufs=3))
        out_pool = ctx.enter_context(tc.tile_pool(name="outsb", bufs=3))
        # PSUM: big(2x2 banks: score pairs / qkv / outproj) + at(4) = 8 banks
        ps_big = ctx.enter_context(
            tc.tile_pool(name="ps_big", bufs=2, space=MemorySpace.PSUM))
        ps_at = ctx.enter_context(
            tc.tile_pool(name="ps_at", bufs=4, space=MemorySpace.PSUM))

        # ------------- persistent SBUF -------------
        # qT of head h in rows 0:64 of qk_sb[h] (rows 64:128 zeroed so score
        # matmuls run K=128 full-row); qk_sb[4+h] = kT of head h.
        qk_sb = [sb.tile([128, S], BF16, tag=f"qk{i}", name=f"qk{i}")
                 for i in range(2 * NH)]
        # v: per seq-tile [128, 4 heads, 65] (64 v cols + ones col)
        v_sb = [sb.tile([128, NH, HD + 1], BF16, tag=f"v{i}", name=f"v{i}")
                for i in range(N_ST)]
        mask_sb = sb.tile([128, 2, 128], F32)   # same causal block, both heads
        owT_sb = [sb.tile([128, D], BF16, tag=f"ow{i}", name=f"ow{i}")
                  for i in range(NP)]
        xT_sb = [sb.tile([128, S], BF16, tag=f"x{i}", name=f"x{i}")
                 for i in range(N_DT)]
        wT_sb = [sb.tile([128, 3 * VW], BF16, tag=f"w{i}", name=f"w{i}")
                 for i in range(N_DT)]
        warm_sb = sb.tile([128, 512], BF16)

        # HAM warm-up: dependency-free matmuls run while the input DMAs
        # stream, so the PE clock-gate is already 8/8 when real work starts.
        nc.vector.memset(warm_sb, 0.0)
        for _ in range(N_WARM):
            wu_ps = ps_big.tile([128, 2, 512], F32, tag="big", name="wu_ps")
            nc.tensor.matmul(wu_ps[:, 0, :], warm_sb[:, 0:128], warm_sb,
                             start=True, stop=True)

        # ------------- input DMAs, priority-ordered -------------
        nc.sync.dma_start(out=mask_sb, in_=mask_d)
        for p in range(NP):
            nc.sync.dma_start(out=owT_sb[p], in_=owT_d[p * 128:(p + 1) * 128, :])
        # first: wq|wk halves + x chunk 0 (unlocks qkv sch=0)
        for d in range(N_DT):
            nc.sync.dma_start(out=wT_sb[d][:, 0:2 * VW],
                              in_=wT_d[d * 128:(d + 1) * 128, 0:2 * VW])
            nc.sync.dma_start(out=xT_sb[d][:, 0:512],
                              in_=xT_d[d * 128:(d + 1) * 128, 0:512])
        for d in range(N_DT):
            nc.sync.dma_start(out=wT_sb[d][:, 2 * VW:3 * VW],
                              in_=wT_d[d * 128:(d + 1) * 128, 2 * VW:3 * VW])
            nc.sync.dma_start(out=xT_sb[d][:, 512:1024],
                              in_=xT_d[d * 128:(d + 1) * 128, 512:1024])
        for sch in range(2, N_CH):
            for d in range(N_DT):
                nc.sync.dma_start(
                    out=xT_sb[d][:, sch * 512:(sch + 1) * 512],
                    in_=xT_d[d * 128:(d + 1) * 128, sch * 512:(sch + 1) * 512])

        # zero-init work that has no input deps (overlaps the DMA wait)
        for t in qk_sb:
            nc.gpsimd.memset(t[HD:128, :], 0.0)
        for t in v_sb:
            nc.gpsimd.memset(t[:, :, HD:HD + 1], 1.0)

        # ------------- qkv projection, one 512-col chunk at a time -------------
        def qkv_sch(sch):
            # early chunks' PSUM->SBUF copies go to Scalar (idle until the
            # first exp), later ones to DVE (Scalar is busy with exp by then)
            eng = nc.scalar.copy if sch < 2 else (
                lambda o, i: nc.vector.tensor_copy(o, i))
            csl = slice(sch * 512, (sch + 1) * 512)
            for f in range(FQK):
                pss = ps_big.tile([128, 2, 512], F32, tag="big", name="psqk")
                for d in range(N_DT):
                    nc.tensor.matmul(
                        pss[:, 0, :],
                        wT_sb[d][:, f * 128:(f + 1) * 128],
                        xT_sb[d][:, csl],
                        start=(d == 0),
                        stop=(d == N_DT - 1),
                    )
                for hh in range(2):
                    eng(qk_sb[2 * f + hh][0:HD, csl],
                        pss[:, 0, :][hh * HD:(hh + 1) * HD, :])
            for st in range(4 * sch, 4 * sch + 4):
                psv = ps_big.tile([128, 2, 512], F32, tag="big", name="psv")
                for d in range(N_DT):
                    nc.tensor.matmul(
                        psv[:, 0, 0:VW],
                        xT_sb[d][:, st * 128:(st + 1) * 128],
                        wT_sb[d][:, 2 * VW:3 * VW],
                        start=(d == 0),
                        stop=(d == N_DT - 1),
                    )
                eng(v_sb[st][:, :, 0:HD],
                    psv[:, 0, 0:VW].rearrange("p (h d) -> p h d", h=NH))

        # ------------- attention -------------
        at_hist = {}   # qc -> [[at_ps x2] x NP]
        an_hist = {}   # qc -> [an x NP]

        def emit_attention(qc):
            n_kt = 4 * (qc + 1)
            ats = []
            for p in range(NP):
                at_ps = [ps_at.tile([HD + 1, 512], F32, tag="at", name="at_ps")
                         for _ in range(2)]
                ats.append(at_ps)
                # one-step software pipeline: scores/exp for kt overlap the
                # attnT accumulation of kt-1, so the in-order PE never waits
                # on the ACT exp chain.
                pend = {}
                DEPTH = 1
                for kt in range(n_kt + DEPTH):
                    if kt < n_kt:
                        j = kt - 4 * qc  # >=0 on diagonal-crossing tiles
                        rs = 0 if j < 0 else j * 128
                        n = 512 - rs
                        st2 = ps_big.tile([128, 2, 512], F32, tag="big",
                                          name="st2")
                        for hp in range(2):
                            nc.tensor.matmul(
                                st2[:, hp, 0:n],
                                qk_sb[NH + 2 * p + hp][:, kt * 128:(kt + 1) * 128],
                                qk_sb[2 * p + hp][:, qc * 512 + rs:(qc + 1) * 512],
                                start=True,
                                stop=True,
                            )
                        if j >= 0:
                            # additive triangular mask on the 128 cols that
                            # cross the diagonal, both heads in one op
                            nc.vector.tensor_add(
                                st2[:, :, 0:128], st2[:, :, 0:128], mask_sb)
                        p2 = p_pool.tile([128, 2, 512], BF16, tag="p")
                        nc.scalar.activation(p2[:, :, 0:n], st2[:, :, 0:n], EXP)
                        pend[kt] = (p2, rs, n)
                    if kt >= DEPTH:
                        p2, rs, n = pend.pop(kt - DEPTH)
                        for hp in range(2):
                            nc.tensor.matmul(
                                at_ps[hp][:, rs:512],
                                v_sb[kt - DEPTH][:, 2 * p + hp, :],
                                p2[:, hp, 0:n],
                                start=(kt == DEPTH),
                                stop=(kt == n_kt + DEPTH - 1),
                            )
            at_hist[qc] = ats

        def emit_normalize(qc):
            ats = at_hist.pop(qc)
            an = [an_pool.tile([128, 512], BF16, tag=f"an{p}", name=f"an{p}")
                  for p in range(NP)]
            an_hist[qc] = an
            for p in range(NP):
                for hp in range(2):
                    at_ps = ats[p][hp]
                    lsb = sm_pool.tile([1, 512], F32, tag="lsb")
                    nc.vector.tensor_copy(lsb, at_ps[HD:HD + 1, :])
                    rec = sm_pool.tile([1, 512], F32, tag="rec")
                    nc.vector.reciprocal_approx_fast(rec, lsb)
                    bc = bc_pool.tile([HD, 512], F32, tag="bc")
                    nc.gpsimd.partition_broadcast(bc, rec)
                    nc.vector.tensor_mul(
                        an[p][hp * HD:(hp + 1) * HD, :], at_ps[0:HD, :], bc)

        def emit_outproj(qc, split=False):
            an = an_hist.pop(qc)
            for qs in range(4):
                qsl = slice(qs * 128, (qs + 1) * 128)
                ops = ps_big.tile([128, 2, 512], F32, tag="big", name="ops")
                for e in range(2):
                    for p in range(NP):
                        nc.tensor.matmul(
                            ops[:, e, :],
                            an[p][:, qsl],
                            owT_sb[p][:, e * 512:(e + 1) * 512],
                            start=(p == 0),
                            stop=(p == NP - 1),
                        )
                osb = out_pool.tile([128, 2, 512], F32, tag="osb", name="osb")
                if split and qs % 2 == 0:
                    nc.scalar.copy(osb, ops)
                else:
                    nc.vector.tensor_copy(osb, ops)
                nc.sync.dma_start(
                    out=out_d[qc * 512 + qs * 128:qc * 512 + (qs + 1) * 128, :],
                    in_=osb.rearrange("p a b -> p (a b)"),
                )

        # Interleaved schedule (see module docstring).
        qkv_sch(0)
        qkv_sch(1)
        emit_attention(1)
        qkv_sch(2)
        emit_normalize(1)
        qkv_sch(3)
        emit_outproj(1)
        emit_attention(2)
        emit_normalize(2)
        emit_attention(3)
        emit_outproj(2)
        emit_normalize(3)
        emit_attention(0)
        emit_normalize(0)
        emit_outproj(3, split=True)
        emit_outproj(0, split=True)


_CACHE = {}


def _build():
    if "nc" in _CACHE:
        return _CACHE["nc"]
    nc = bacc.Bacc("TRN2", target_bir_lowering=False, debug=False)
    xT_d = nc.dram_tensor("xT", [D, S], BF16, kind="ExternalInput").ap()
    wT_d = nc.dram_tensor("wT", [D, 3 * VW], BF16, kind="ExternalInput").ap()
    owT_d = nc.dram_tensor("owT", [VW, D], BF16, kind="ExternalInput").ap()
    mask_d = nc.dram_tensor("mask", [128, 2, 128], F32, kind="ExternalInput").ap()
    out_d = nc.dram_tensor("out", [S, D], F32, kind="ExternalOutput").ap()
    with tile.TileContext(nc) as tc:
        _emit(tc, nc, xT_d, wT_d, owT_d, mask_d, out_d)
    nc.compile()
    _CACHE["nc"] = nc
    return nc


def _mask_np():
    # [128, 2, 128] additive causal block for the diagonal-crossing tile,
    # laid out [k, head, q]: keep (0.0) where q >= k else NEG; identical for
    # both heads of a pair (the batched mask add covers a 2-bank score tile).
    r = np.arange(128)
    tri = np.where(r[None, :] >= r[:, None], 0.0, NEG).astype(np.float32)
    return np.ascontiguousarray(np.stack([tri, tri], axis=1))


def make_in_maps(x, qkv_w, out_w):
    """Per-core input dicts for the 8-way (batch x head-group) sharding."""
    x = np.asarray(x, np.float32)
    qkv_w = np.asarray(qkv_w, np.float32)
    out_w = np.asarray(out_w, np.float32)
    xT = [np.ascontiguousarray(x[b].T) for b in range(B)]
    mask = _mask_np()
    import ml_dtypes
    np_mm = ml_dtypes.bfloat16
    in_maps = []
    for c in range(NCORES):
        b = c // 4
        h0 = (c % 4) * NH
        rows = np.arange(h0 * HD, (h0 + NH) * HD)
        wq = qkv_w[rows] * np.float32(SCALE)
        wk = qkv_w[D + rows]
        wv = qkv_w[2 * D + rows]
        wT = np.ascontiguousarray(np.concatenate([wq, wk, wv], 0).T)
        owT = np.ascontiguousarray(out_w[:, rows].T)
        in_maps.append({"xT": xT[b].astype(np_mm), "wT": wT.astype(np_mm),
                        "owT": owT.astype(np_mm), "mask": mask})
    return in_maps


def kernel(x, qkv_w, out_w, _trace=False, _trace_cores=None):
    nc = _build()
    in_maps = make_in_maps(x, qkv_w, out_w)
    res = run_bass_kernel_spmd(
        nc, in_maps, core_ids=list(range(NCORES)),
        trace=_trace, trace_cores=_trace_cores,
    )
    outs = [r["out"] for r in res.results]
    full = np.stack([
        outs[0] + outs[1] + outs[2] + outs[3],
        outs[4] + outs[5] + outs[6] + outs[7],
    ]).astype(np.float32)
    if _trace:
        return full, res
    return full


# revision 11
# speedup vs baseline: 1.0426x; 1.0426x over previous
"""Causal multi-head attention block (qkv proj + causal softmax attention + out proj)
for Trainium2, sharded over 8 NeuronCores: data-parallel over batch (2) x
tensor-parallel over heads (4 heads per core of 16).

Each core computes, for its batch b and its 4 heads:
  qT,kT [hd, S] and v [S, hd]  (qkv projection, weights pre-transposed on host)
  ST    [k, 2, q] = scores of a head pair, causal-blocked (one 2-bank PSUM tile)
  P     = exp(ST + mask)   (one ACT instruction per k-tile covers both heads)
  attnT [hd+1, q] = [v | 1].T @ P   (row hd = softmax denominator)
  an    = attnT / denom
  out_partial [S, D] = an.T @ owT  (row-parallel out proj)
Host sums the 4 per-core partials of each batch.

Emission is interleaved so input DMA, qkv projection, attention and
out-projection overlap: x is DMA'd in 512-column chunks and each qkv chunk
is computed as its chunk lands; attention chunk qc is emitted as soon as
its kT/qT/v tiles exist; out-proj of chunk qc is deferred past the next
attention chunk so its matmuls never wait on the normalize chain; the
chunk order ends on the shortest chunk (qc=0) to minimize the tail.
"""

import contextlib
import sys

import numpy as np

sys.path.insert(0, "/opt/trn_rl_repo")

import concourse.bass as bass
import concourse.tile as tile
from concourse import bacc, mybir
from concourse.bass import MemorySpace
from concourse.bass_utils import run_bass_kernel_spmd

F32 = mybir.dt.float32
BF16 = mybir.dt.bfloat16
EXP = mybir.ActivationFunctionType.Exp

B, S, D = 2, 2048, 1024
H, HD = 16, 64
NCORES = 8
NH = 4          # heads per core
NP = 2          # head pairs per core
SCALE = HD ** -0.5

N_DT = D // 128          # 8 d-tiles of 128
N_ST = S // 128          # 16 seq tiles of 128
N_CH = S // 512          # 4 seq chunks of 512
FQK = 2 * NH * HD // 128  # 4 f-tiles covering q|k (pair-major)
VW = NH * HD             # 256 v columns
NEG = -1.0e9
N_WARM = 16


def _emit(tc, nc, xT_d, wT_d, owT_d, mask_d, out_d):
    ctx = contextlib.ExitStack()
    with ctx:
        # ------------- pools -------------
        sb = ctx.enter_context(tc.tile_pool(name="sb", bufs=1))
        p_pool = ctx.enter_context(tc.tile_pool(name="psb", bufs=3))
        an_pool = ctx.enter_context(tc.tile_pool(name="attn_n", bufs=2))
        sm_pool = ctx.enter_context(tc.tile_pool(name="smalls", bufs=4))
        bc_pool = ctx.enter_context(tc.tile_pool(name="bcast", bufs=3))
        out_pool = ctx.enter_context(tc.tile_pool(name="outsb", bufs=3))
        # PSUM: big(3x2 banks: score pairs / qkv / outproj) + at(2) = 8 banks
        ps_big = ctx.enter_context(
            tc.tile_pool(name="ps_big", bufs=3, space=MemorySpace.PSUM))
        ps_at = ctx.enter_context(
            tc.tile_pool(name="ps_at", bufs=2, space=MemorySpace.PSUM))

        # ------------- persistent SBUF -------------
        # qT of head h in rows 0:64 of qk_sb[h] (rows 64:128 zeroed so score
        # matmuls run K=128 full-row); qk_sb[4+h] = kT of head h.
        qk_sb = [sb.tile([128, S], BF16, tag=f"qk{i}", name=f"qk{i}")
                 for i in range(2 * NH)]
        # v: per seq-tile [128, 4 heads, 65] (64 v cols + ones col)
        v_sb = [sb.tile([128, NH, HD + 1], BF16, tag=f"v{i}", name=f"v{i}")
                for i in range(N_ST)]
        mask_sb = sb.tile([128, 2, 128], F32)   # same causal block, both heads
        owT_sb = [sb.tile([128, D], BF16, tag=f"ow{i}", name=f"ow{i}")
                  for i in range(NP)]
        xT_sb = [sb.tile([128, S], BF16, tag=f"x{i}", name=f"x{i}")
                 for i in range(N_DT)]
        wT_sb = [sb.tile([128, 3 * VW], BF16, tag=f"w{i}", name=f"w{i}")
                 for i in range(N_DT)]
        warm_sb = sb.tile([128, 512], BF16)

        # HAM warm-up: dependency-free matmuls run while the input DMAs
        # stream, so the PE clock-gate is already 8/8 when real work starts.
        nc.vector.memset(warm_sb, 0.0)
        for _ in range(N_WARM):
            wu_ps = ps_big.tile([128, 2, 512], F32, tag="big", name="wu_ps")
            nc.tensor.matmul(wu_ps[:, 0, :], warm_sb[:, 0:128], warm_sb,
                             start=True, stop=True)

        # ------------- input DMAs, priority-ordered -------------
        # Issue on BOTH hwdge engines (sync + scalar) in parallel: a single
        # engine needs ~600ns per dma_start, which would serialize the input
        # stream behind the issue rate.
        nc.sync.dma_start(out=mask_sb, in_=mask_d)
        for p in range(NP):
            nc.scalar.dma_start(out=owT_sb[p], in_=owT_d[p * 128:(p + 1) * 128, :])
        # first: wq|wk halves + x chunk 0 (unlocks qkv sch=0)
        for d in range(N_DT):
            nc.sync.dma_start(out=wT_sb[d][:, 0:2 * VW],
                              in_=wT_d[d * 128:(d + 1) * 128, 0:2 * VW])
            nc.scalar.dma_start(out=xT_sb[d][:, 0:512],
                                in_=xT_d[d * 128:(d + 1) * 128, 0:512])
        for d in range(N_DT):
            nc.sync.dma_start(out=wT_sb[d][:, 2 * VW:3 * VW],
                              in_=wT_d[d * 128:(d + 1) * 128, 2 * VW:3 * VW])
            nc.scalar.dma_start(out=xT_sb[d][:, 512:1024],
                                in_=xT_d[d * 128:(d + 1) * 128, 512:1024])
        for d in range(N_DT):
            eng = nc.sync if d % 2 == 0 else nc.scalar
            eng.dma_start(
                out=xT_sb[d][:, 1024:2048],
                in_=xT_d[d * 128:(d + 1) * 128, 1024:2048])

        # zero-init work that has no input deps (overlaps the DMA wait)
        for t in qk_sb:
            nc.gpsimd.memset(t[HD:128, :], 0.0)
        for t in v_sb:
            nc.gpsimd.memset(t[:, :, HD:HD + 1], 1.0)

        # ------------- qkv projection, one 512-col chunk at a time -------------
        def qkv_sch(sch):
            # early chunks' PSUM->SBUF copies go to Scalar (idle until the
            # first exp), later ones to DVE (Scalar is busy with exp by then)
            eng = nc.scalar.copy if sch < 2 else (
                lambda o, i: nc.vector.tensor_copy(o, i))
            csl = slice(sch * 512, (sch + 1) * 512)
            for f in range(FQK):
                pss = ps_big.tile([128, 2, 512], F32, tag="big", name="psqk")
                for d in range(N_DT):
                    nc.tensor.matmul(
                        pss[:, 0, :],
                        wT_sb[d][:, f * 128:(f + 1) * 128],
                        xT_sb[d][:, csl],
                        start=(d == 0),
                        stop=(d == N_DT - 1),
                    )
                for hh in range(2):
                    eng(qk_sb[2 * f + hh][0:HD, csl],
                        pss[:, 0, :][hh * HD:(hh + 1) * HD, :])
            for st in range(4 * sch, 4 * sch + 4):
                psv = ps_big.tile([128, 2, 512], F32, tag="big", name="psv")
                for d in range(N_DT):
                    nc.tensor.matmul(
                        psv[:, 0, 0:VW],
                        xT_sb[d][:, st * 128:(st + 1) * 128],
                        wT_sb[d][:, 2 * VW:3 * VW],
                        start=(d == 0),
                        stop=(d == N_DT - 1),
                    )
                eng(v_sb[st][:, :, 0:HD],
                    psv[:, 0, 0:VW].rearrange("p (h d) -> p h d", h=NH))

        # ------------- attention -------------
        an_hist = {}   # qc -> [an x NP]

        def emit_attention(qc):
            n_kt = 4 * (qc + 1)
            an = [an_pool.tile([128, 512], BF16, tag=f"an{p}", name=f"an{p}")
                  for p in range(NP)]
            an_hist[qc] = an
            for p in range(NP):
                at_ps = [ps_at.tile([HD + 1, 512], F32, tag="at", name="at_ps")
                         for _ in range(2)]
                # two-step software pipeline: scores/exp for kt overlap the
                # attnT accumulation of kt-2, so the in-order PE never waits
                # on the ACT exp chain or its semaphore hops.
                pend = {}
                DEPTH = 2
                for kt in range(n_kt + DEPTH):
                    if kt < n_kt:
                        j = kt - 4 * qc  # >=0 on diagonal-crossing tiles
                        rs = 0 if j < 0 else j * 128
                        n = 512 - rs
                        st2 = ps_big.tile([128, 2, 512], F32, tag="big",
                                          name="st2")
                        for hp in range(2):
                            nc.tensor.matmul(
                                st2[:, hp, 0:n],
                                qk_sb[NH + 2 * p + hp][:, kt * 128:(kt + 1) * 128],
                                qk_sb[2 * p + hp][:, qc * 512 + rs:(qc + 1) * 512],
                                start=True,
                                stop=True,
                            )
                        if j >= 0:
                            # additive triangular mask on the 128 cols that
                            # cross the diagonal, both heads in one op
                            nc.vector.tensor_add(
                                st2[:, :, 0:128], st2[:, :, 0:128], mask_sb)
                        p2 = p_pool.tile([128, 2, 512], BF16, tag="p")
                        nc.scalar.activation(p2[:, :, 0:n], st2[:, :, 0:n], EXP)
                        pend[kt] = (p2, rs, n)
                    kc = kt - DEPTH
                    if 0 <= kc < n_kt:
                        p2, rs, n = pend.pop(kc)
                        for hp in range(2):
                            nc.tensor.matmul(
                                at_ps[hp][:, rs:512],
                                v_sb[kc][:, 2 * p + hp, :],
                                p2[:, hp, 0:n],
                                start=(kc == 0),
                                stop=(kc == n_kt - 1),
                            )
                # normalize this pair right away: the chain runs on DVE/GpSimd
                # while the PE streams the next pair's scores, and it frees
                # the two `at` PSUM banks for that pair.
                for hp in range(2):
                    lsb = sm_pool.tile([1, 512], F32, tag="lsb")
                    nc.vector.tensor_copy(lsb, at_ps[hp][HD:HD + 1, :])
                    rec = sm_pool.tile([1, 512], F32, tag="rec")
                    nc.vector.reciprocal_approx_fast(rec, lsb)
                    bc = bc_pool.tile([HD, 512], F32, tag="bc")
                    nc.gpsimd.partition_broadcast(bc, rec)
                    nc.vector.tensor_mul(
                        an[p][hp * HD:(hp + 1) * HD, :], at_ps[hp][0:HD, :], bc)

        def emit_outproj(qc, split=False):
            an = an_hist.pop(qc)
            for qs in range(4):
                qsl = slice(qs * 128, (qs + 1) * 128)
                ops = ps_big.tile([128, 2, 512], F32, tag="big", name="ops")
                for e in range(2):
                    for p in range(NP):
                        nc.tensor.matmul(
                            ops[:, e, :],
                            an[p][:, qsl],
                            owT_sb[p][:, e * 512:(e + 1) * 512],
                            start=(p == 0),
                            stop=(p == NP - 1),
                        )
                osb = out_pool.tile([128, 2, 512], BF16, tag="osb", name="osb")
                if split and qs % 2 == 0:
                    nc.scalar.copy(osb, ops)
                else:
                    nc.vector.tensor_copy(osb, ops)
                nc.sync.dma_start(
                    out=out_d[qc * 512 + qs * 128:qc * 512 + (qs + 1) * 128, :],
                    in_=osb.rearrange("p a b -> p (a b)"),
                )

        # Interleaved schedule (see module docstring).
        qkv_sch(0)
        qkv_sch(1)
        emit_attention(1)
        qkv_sch(2)
        qkv_sch(3)
        emit_outproj(1)
        emit_attention(2)
        emit_attention(3)
        emit_outproj(2)
        emit_attention(0)
        emit_outproj(3, split=True)
        emit_outproj(0, split=True)


_CACHE = {}


def _build():
    if "nc" in _CACHE:
        return _CACHE["nc"]
    nc = bacc.Bacc("TRN2", target_bir_lowering=False, debug=False)
    xT_d = nc.dram_tensor("xT", [D, S], BF16, kind="ExternalInput").ap()
    wT_d = nc.dram_tensor("wT", [D, 3 * VW], BF16, kind="ExternalInput").ap()
    owT_d = nc.dram_tensor("owT", [VW, D], BF16, kind="ExternalInput").ap()
    mask_d = nc.dram_tensor("mask", [128, 2, 128], F32, kind="ExternalInput").ap()
    out_d = nc.dram_tensor("out", [S, D], BF16, kind="ExternalOutput").ap()
    with tile.TileContext(nc) as tc:
        _emit(tc, nc, xT_d, wT_d, owT_d, mask_d, out_d)
    nc.compile()
    _CACHE["nc"] = nc
    return nc


def _mask_np():
    # [128, 2, 128] additive causal block for the diagonal-crossing tile,
    # laid out [k, head, q]: keep (0.0) where q >= k else NEG; identical for
    # both heads of a pair (the batched mask add covers a 2-bank score tile).
    r = np.arange(128)
    tri = np.where(r[None, :] >= r[:, None], 0.0, NEG).astype(np.float32)
    return np.ascontiguousarray(np.stack([tri, tri], axis=1))


def make_in_maps(x, qkv_w, out_w):
    """Per-core input dicts for the 8-way (batch x head-group) sharding."""
    x = np.asarray(x, np.float32)
    qkv_w = np.asarray(qkv_w, np.float32)
    out_w = np.asarray(out_w, np.float32)
    xT = [np.ascontiguousarray(x[b].T) for b in range(B)]
    mask = _mask_np()
    import ml_dtypes
    np_mm = ml_dtypes.bfloat16
    in_maps = []
    for c in range(NCORES):
        b = c // 4
        h0 = (c % 4) * NH
        rows = np.arange(h0 * HD, (h0 + NH) * HD)
        wq = qkv_w[rows] * np.float32(SCALE)
        wk = qkv_w[D + rows]
        wv = qkv_w[2 * D + rows]
        wT = np.ascontiguousarray(np.concatenate([wq, wk, wv], 0).T)
        owT = np.ascontiguousarray(out_w[:, rows].T)
        in_maps.append({"xT": xT[b].astype(np_mm), "wT": wT.astype(np_mm),
                        "owT": owT.astype(np_mm), "mask": mask})
    return in_maps


def kernel(x, qkv_w, out_w, _trace=False, _trace_cores=None):
    nc = _build()
    in_maps = make_in_maps(x, qkv_w, out_w)
    res = run_bass_kernel_spmd(
        nc, in_maps, core_ids=list(range(NCORES)),
        trace=_trace, trace_cores=_trace_cores,
    )
    outs = [np.asarray(r["out"], np.float32) for r in res.results]
    full = np.stack([
        outs[0] + outs[1] + outs[2] + outs[3],
        outs[4] + outs[5] + outs[6] + outs[7],
    ]).astype(np.float32)
    if _trace:
        return full, res
    return full
